# revision 9
# baseline (speedup 1.0000x reference)
"""Trainium2 Bass kernel for a 2-layer GCN (PyG GCNConv x2 with self-loops).

Reference (N=100000 nodes, E=1600000 edges, f32):
    A = D^-1/2 (Adj + I) D^-1/2
    h   = relu(A x W1 + b1)
    out = A h W2 + b2

Key factorization: A[d,s] = dis[d]*dis[s] (dis = deg^-1/2, deg incl the
self loop).  The host pre-scales x rows by dis[src]; the dis[dst] factor
is folded into the gathered rows by a per-chunk scalar-engine scale
(per-partition = per-edge there), so the selection matrix P is a pure
one-hot built with a single vector op per gather tile and no per-edge
norm multiply is needed on the vector engine.

Self-loop (+I) terms never enter the edge stream: layer 1 initializes the
accumulator with a host-transposed x*dis^2 shard; layer 2 DMA-transposes
a h*dis^2 copy written during the layer-1 dense phase.

Distribution: destination sharding (12500 nodes/core), WIN=512 dest
windows, edges bucketed host-side by (source-chunk k, window w), padded
to 128-edge chunks equalized across cores (SPMD).  Per chunk the tensor
engine computes PSUM[feat, 512] += gt_scaled[:,slot,:]^T @ P.  Layer-1
output h (scaled by dis) is AllGathered in 4 window-aligned chunks which
double as layer-2 source chunks; each AllGather fires as soon as its
windows finish, and the layer-2 dense transform + output stores stream
inside the last aggregation pass.
"""

import os
import sys

import numpy as np

for _p in ("/opt/trn_rl_repo", "/root/.axon_site/_ro/trn_rl_repo"):
    if os.path.isdir(_p) and _p not in sys.path:
        sys.path.insert(0, _p)

# ----------------------------------------------------------------------------
# Problem constants
# ----------------------------------------------------------------------------
N = 100000
NC = 8
NS = N // NC            # 12500 dest nodes per core
D0, D1, D2 = 64, 128, 256
WIN = 512               # dest window width
NW = (NS + WIN - 1) // WIN          # 25 windows per core
ACCW = NW * WIN                     # 12800 acc columns
NSRC = 4                # source chunks per layer
SC1 = N // NSRC         # 25000 rows per layer-1 source chunk (src % 4)
# layer-2 / AllGather chunks: window-aligned [6,6,6,7] windows
AGW = [6, 6, 6, 7]
AGSZ = [6 * WIN, 6 * WIN, 6 * WIN, NS - 18 * WIN]   # 3072,3072,3072,3284
AGOFF = [0, 3072, 6144, 9216]
H3PAD = 3296            # h2_own_3 rows padded to 16-multiple for dma transpose
G = 1024                # edges per dma_gather call (SWDGE ring capacity)
GC = G // 128
NQ = 4
MCHUNKS = (NS + 127) // 128         # 98 dense row-chunks


# ----------------------------------------------------------------------------
# Host-side preprocessing
# ----------------------------------------------------------------------------
def _plan_layer(src_all, dloc_all, dd_all, core_all, src_chunk_of, src_idx_of,
                dd_dtype=np.float32):
    """Bucket per-core edges by (source-chunk k, dest-window w), pad each
    bucket to a 128-multiple equal across cores.  Returns (meta, per_core):
      meta = {"Tk": [...], "segs": [[(w, nchunks), ...] per k]}
      per_core[c] = {"idx": int16 [128, T/16] x4, "dst": f16 [128, T/128] x4,
                     "dd": f16 [128, T/128] x4 (dis[dst], 0 on padding)}
    """
    counts = np.zeros((NC, NSRC, NW), dtype=np.int64)
    per_core = []
    for c in range(NC):
        sel = core_all == c
        src = src_all[sel]
        dloc = dloc_all[sel]
        dd = dd_all[sel]
        k = src_chunk_of(src)
        w = dloc // WIN
        idxl = src_idx_of(src)
        order = np.lexsort((w, k))
        k, w, idxl, dloc, dd = k[order], w[order], idxl[order], dloc[order], dd[order]
        key = k * NW + w
        counts[c] = np.bincount(key, minlength=NSRC * NW).reshape(NSRC, NW)
        per_core.append((k, w, idxl, dloc, dd, key))

    nch = (counts.max(axis=0) + 127) // 128
    Tk = (nch.sum(axis=1) * 128).astype(np.int64)
    segs = [[(int(w), int(nch[k, w])) for w in range(NW) if nch[k, w] > 0]
            for k in range(NSRC)]

    base = np.zeros((NSRC, NW), dtype=np.int64)
    for k in range(NSRC):
        base[k] = np.concatenate(([0], np.cumsum(nch[k] * 128)[:-1]))

    out = []
    for c in range(NC):
        k, w, idxl, dloc, dd, key = per_core[c]
        cnt = counts[c].reshape(-1)
        starts = np.concatenate(([0], np.cumsum(cnt)[:-1]))
        pos_in_bucket = np.arange(len(key)) - starts[key]
        tgt = base.reshape(-1)[key] + pos_in_bucket
        arrs = {"idx": [], "dst": [], "dd": []}
        for kk in range(NSRC):
            T = int(Tk[kk])
            idx16 = np.zeros(T, dtype=np.int16)
            dwf = np.full(T, -1.0, dtype=np.float16)
            ddf = np.zeros(T, dtype=dd_dtype)
            m = k == kk
            t = tgt[m]
            idx16[t] = idxl[m].astype(np.int16)
            dwf[t] = (dloc[m] - (w[m] * WIN)).astype(np.float16)
            ddf[t] = dd[m].astype(dd_dtype)
            arrs["idx"].append(np.ascontiguousarray(
                np.tile(idx16.reshape(T // 16, 16).T, (8, 1))))
            arrs["dst"].append(np.ascontiguousarray(dwf.reshape(T // 128, 128).T))
            arrs["dd"].append(np.ascontiguousarray(ddf.reshape(T // 128, 128).T))
        out.append(arrs)
    return {"Tk": [int(t) for t in Tk], "segs": segs}, out


def _preprocess(x, edge_index, W1, b1, W2, b2):
    row = np.asarray(edge_index[0], dtype=np.int64)
    col = np.asarray(edge_index[1], dtype=np.int64)
    deg = (np.bincount(col, minlength=N) + 1).astype(np.float32)
    dis = (1.0 / np.sqrt(deg)).astype(np.float32)

    core = (col // NS).astype(np.int64)
    dloc = col - core * NS
    ddst = dis[col]

    meta1, arrs1 = _plan_layer(
        row, dloc, ddst, core,
        src_chunk_of=lambda s: s % NSRC,
        src_idx_of=lambda s: s // NSRC,
    )

    def chunk2_of(s):
        return np.minimum((s % NS) // AGSZ[0], 3)

    def idx2_of(s):
        c = s // NS
        r = s % NS
        q = np.minimum(r // AGSZ[0], 3)
        szq = np.asarray(AGSZ, dtype=np.int64)[q]
        offq = np.asarray(AGOFF, dtype=np.int64)[q]
        return c * szq + (r - offq)

    meta2, arrs2 = _plan_layer(row, dloc, ddst, core, chunk2_of, idx2_of,
                               dd_dtype=np.float16)

    x = np.asarray(x, dtype=np.float32)
    x_dis = np.ascontiguousarray(x * dis[:, None])

    shared = {
        "x_dis": x_dis,
        "W1": np.ascontiguousarray(np.asarray(W1, dtype=np.float32)),
        "b1": np.ascontiguousarray(np.asarray(b1, dtype=np.float32).reshape(1, D1)),
        "W2": np.ascontiguousarray(np.asarray(W2, dtype=np.float32)),
        "b2": np.ascontiguousarray(np.asarray(b2, dtype=np.float32).reshape(1, D2)),
    }
    in_maps = []
    for c in range(NC):
        m = dict(shared)
        lo, hi = c * NS, (c + 1) * NS
        m["xT"] = np.ascontiguousarray((x_dis[lo:hi] * dis[lo:hi, None]).T)
        dv = np.zeros(128 * MCHUNKS, dtype=np.float32)
        dv[:NS] = dis[lo:hi]
        m["disw"] = np.ascontiguousarray(dv.reshape(MCHUNKS, 128).T)  # [128, 98]
        for kk in range(NSRC):
            m[f"idx1_{kk}"] = arrs1[c]["idx"][kk]
            m[f"dst1_{kk}"] = arrs1[c]["dst"][kk]
            m[f"dd1_{kk}"] = arrs1[c]["dd"][kk]
            m[f"idx2_{kk}"] = arrs2[c]["idx"][kk]
            m[f"dst2_{kk}"] = arrs2[c]["dst"][kk]
            m[f"dd2_{kk}"] = arrs2[c]["dd"][kk]
        in_maps.append(m)
    return meta1, meta2, in_maps


# ----------------------------------------------------------------------------
# Device program
# ----------------------------------------------------------------------------
def _build(meta1, meta2, debug=False, dbg_stages=False):
    from contextlib import ExitStack

    import concourse.bacc as bacc
    import concourse.bass as bass
    import concourse.mybir as mybir
    import concourse.tile as tile

    f32, f16, i16 = mybir.dt.float32, mybir.dt.float16, mybir.dt.int16
    Relu = mybir.ActivationFunctionType.Relu
    Copy = mybir.ActivationFunctionType.Copy

    nc = bacc.Bacc("TRN2", target_bir_lowering=False, debug=debug,
                   num_devices=NC, num_swdge_queues=NQ)

    x_d = nc.dram_tensor("x_dis", [N, D0], f32, kind="ExternalInput")
    xT_d = nc.dram_tensor("xT", [D0, NS], f32, kind="ExternalInput")
    disw_d = nc.dram_tensor("disw", [128, MCHUNKS], f32, kind="ExternalInput")
    w1_d = nc.dram_tensor("W1", [D0, D1], f32, kind="ExternalInput")
    b1_d = nc.dram_tensor("b1", [1, D1], f32, kind="ExternalInput")
    w2_d = nc.dram_tensor("W2", [D1, D2], f32, kind="ExternalInput")
    b2_d = nc.dram_tensor("b2", [1, D2], f32, kind="ExternalInput")

    idx1_d, dst1_d, dd1_d, idx2_d, dst2_d, dd2_d = [], [], [], [], [], []
    for k in range(NSRC):
        T1, T2 = meta1["Tk"][k], meta2["Tk"][k]
        idx1_d.append(nc.dram_tensor(f"idx1_{k}", [128, T1 // 16], i16, kind="ExternalInput"))
        dst1_d.append(nc.dram_tensor(f"dst1_{k}", [128, T1 // 128], f16, kind="ExternalInput"))
        dd1_d.append(nc.dram_tensor(f"dd1_{k}", [128, T1 // 128], f32, kind="ExternalInput"))
        idx2_d.append(nc.dram_tensor(f"idx2_{k}", [128, T2 // 16], i16, kind="ExternalInput"))
        dst2_d.append(nc.dram_tensor(f"dst2_{k}", [128, T2 // 128], f16, kind="ExternalInput"))
        dd2_d.append(nc.dram_tensor(f"dd2_{k}", [128, T2 // 128], f16, kind="ExternalInput"))

    h_own = [nc.dram_tensor(f"h_own{q}", [AGSZ[q], D1], f16, kind="Internal")
             for q in range(4)]
    h2_own = [nc.dram_tensor(f"h2_own{q}", [H3PAD if q == 3 else AGSZ[q], D1],
                             f16, kind="Internal") for q in range(4)]
    hf = [nc.dram_tensor(f"hf{q}", [NC * AGSZ[q], D1], f16, kind="Internal",
                         addr_space="Shared") for q in range(4)]
    out_d = nc.dram_tensor("out", [NS, D2], f32, kind="ExternalOutput")
    if dbg_stages:
        dacc1_d = nc.dram_tensor("dacc1", [D0, ACCW], f32, kind="ExternalOutput")
        dh_d = nc.dram_tensor("dh", [NS, D1], f16, kind="ExternalOutput")
        dacc2_d = nc.dram_tensor("dacc2", [D1, ACCW], f32, kind="ExternalOutput")

    def bcast(col_slice, mc, width=WIN):
        return bass.AP(col_slice.tensor, col_slice.offset,
                       [list(col_slice.ap[0]), [1, mc], [0, width]])

    with tile.TileContext(nc) as tc:
        with ExitStack() as top:
            const = top.enter_context(tc.tile_pool(name="const", bufs=1))
            w1_t = const.tile([D0, D1], f32)
            nc.sync.dma_start(w1_t[:], w1_d[:])
            b1_t = const.tile([1, D1], f32)
            nc.sync.dma_start(b1_t[:], b1_d[:])
            w2_t = const.tile([D1, D2], f32)
            nc.sync.dma_start(w2_t[:], w2_d[:])
            b2_t = const.tile([1, D2], f32)
            nc.sync.dma_start(b2_t[:], b2_d[:])
            disw_t = const.tile([128, MCHUNKS], f32)
            nc.sync.dma_start(disw_t[:], disw_d[:])
            iota16 = const.tile([128, GC, WIN], f16)
            nc.gpsimd.iota(iota16[:], pattern=[[0, GC], [1, WIN]], base=0,
                           channel_multiplier=0,
                           allow_small_or_imprecise_dtypes=True)
            ones_t = const.tile([1, 128], f32)
            nc.vector.memset(ones_t[:], 1.0)

            accp = top.enter_context(tc.tile_pool(name="acc", bufs=1))

            # preload all layer-2 meta on the Activation HWDGE queue
            mp2 = top.enter_context(tc.tile_pool(name="meta2", bufs=1))
            idx2_t, dst2_t, dd2_t = [], [], []
            for k in range(NSRC):
                T2 = meta2["Tk"][k]
                t = mp2.tile([128, T2 // 16], i16, tag=f"idx2_{k}")
                nc.scalar.dma_start(t[:], idx2_d[k][:])
                idx2_t.append(t)
                t = mp2.tile([128, T2 // 128], f16, tag=f"dst2_{k}")
                nc.scalar.dma_start(t[:], dst2_d[k][:])
                dst2_t.append(t)
                t = mp2.tile([128, T2 // 128], f16, tag=f"dd2_{k}")
                nc.scalar.dma_start(t[:], dd2_d[k][:])
                dd2_t.append(t)

            # =========== Layer 1 ===========
            with ExitStack() as l1s:
                acc1p = l1s.enter_context(tc.tile_pool(name="acc1", bufs=1))
                acc1 = acc1p.tile([D0, ACCW], f32)
                nc.vector.memset(acc1[:, NS:], 0.0)
                nc.sync.dma_start(acc1[:, :NS], xT_d[:])   # self-loop init

                mp = l1s.enter_context(tc.tile_pool(name="meta1", bufs=2))
                gp = l1s.enter_context(tc.tile_pool(name="g1", bufs=8))
                pp = l1s.enter_context(tc.tile_pool(name="p1", bufs=2))
                psp = l1s.enter_context(tc.tile_pool(name="ps1", bufs=3, space="PSUM"))
                hp = l1s.enter_context(tc.tile_pool(name="hb", bufs=4))
                psb = l1s.enter_context(tc.tile_pool(name="psb", bufs=2, space="PSUM"))

                xb = x_d[:]
                x_srcs = [bass.AP(xb.tensor, k * D0, [[NSRC * D0, SC1], [1, D0]])
                          for k in range(NSRC)]

                def l1_dense(w):
                    q = min(w // 6, 3)
                    for mm in range(4 * w, min(4 * w + 4, MCHUNKS)):
                        M = min(128, NS - mm * 128)
                        ps2 = psb.tile([M, D1], f32, tag="psb")
                        nc.tensor.matmul(ps2[:], acc1[:, mm * 128:mm * 128 + M],
                                         w1_t[:], start=True, stop=False)
                        nc.tensor.matmul(ps2[:], ones_t[:, :M], b1_t[:],
                                         start=False, stop=True)
                        ht = hp.tile([M, D1], f16, tag="ht")
                        nc.scalar.activation(ht[:], ps2[:], Relu,
                                             scale=disw_t[:M, mm:mm + 1])
                        ht2 = hp.tile([M, D1], f16, tag="ht2")
                        nc.scalar.activation(ht2[:], ht[:], Copy,
                                             scale=disw_t[:M, mm:mm + 1])
                        lo = mm * 128 - AGOFF[q]
                        nc.sync.dma_start(h_own[q][lo:lo + M, :], ht[:])
                        nc.sync.dma_start(h2_own[q][lo:lo + M, :], ht2[:])

                ncalls = 0
                for k in range(NSRC):
                    Tk = meta1["Tk"][k]
                    segd = dict(meta1["segs"][k])
                    idx_t = mp.tile([128, Tk // 16], i16, tag="idx1")
                    nc.scalar.dma_start(idx_t[:], idx1_d[k][:])
                    dst_t = mp.tile([128, Tk // 128], f16, tag="dst1")
                    nc.scalar.dma_start(dst_t[:], dst1_d[k][:])
                    dd_t = mp.tile([128, Tk // 128], f32, tag="dd1")
                    nc.scalar.dma_start(dd_t[:], dd1_d[k][:])
                    jj = 0
                    gt = None
                    gt16 = None
                    P8 = None
                    for w in range(NW):
                        nchk = segd.get(w, 0)
                        if nchk:
                            ps = psp.tile([D0, WIN], f32, tag="ps1")
                            for j in range(nchk):
                                g, slot = divmod(jj, GC)
                                if slot == 0:
                                    mlen = min(G, Tk - g * G)
                                    mc = mlen // 128
                                    gt = gp.tile([128, GC, D0], f32, tag="gt32", bufs=5)
                                    nc.gpsimd.dma_gather(
                                        gt[:, :mc, :], x_srcs[k],
                                        idx_t[:, g * (G // 16): (g * G + mlen) // 16],
                                        mlen, mlen, D0,
                                        elem_step=NSRC * D0,
                                        queue_num=ncalls % NQ,
                                        single_packet=True,
                                    )
                                    ncalls += 1
                                    gt16 = gp.tile([128, GC, D0], f16, tag="gt16")
                                    nc.vector.tensor_tensor(
                                        gt16[:, :mc, :], gt[:, :mc, :],
                                        bcast(dd_t[:, jj:jj + mc], mc, D0),
                                        mybir.AluOpType.mult)
                                    P8 = pp.tile([128, GC, WIN], f16, tag="P1")
                                    nc.vector.tensor_tensor(
                                        P8[:, :mc, :], iota16[:, :mc, :],
                                        bcast(dst_t[:, jj:jj + mc], mc),
                                        mybir.AluOpType.is_equal)
                                nc.tensor.matmul(ps[:], gt16[:, slot, :],
                                                 P8[:, slot, :],
                                                 start=(j == 0), stop=(j == nchk - 1))
                                jj += 1
                            nc.vector.tensor_tensor(
                                acc1[:, w * WIN:(w + 1) * WIN],
                                acc1[:, w * WIN:(w + 1) * WIN], ps[:],
                                mybir.AluOpType.add)
                        if k == NSRC - 1:
                            l1_dense(w)
                for q in range(4):
                    nc.gpsimd.collective_compute(
                        "AllGather", mybir.AluOpType.bypass,
                        replica_groups=[list(range(NC))],
                        ins=[h_own[q][:, :]],
                        outs=[hf[q][:, :]],
                    )
                if dbg_stages:
                    nc.sync.dma_start(dacc1_d[:], acc1[:])
                    for q in range(4):
                        nc.sync.dma_start(
                            dh_d[AGOFF[q]:AGOFF[q] + AGSZ[q], :],
                            h_own[q][:, :])

            # =========== Layer 2 ===========
            acc2 = accp.tile([D1, ACCW], f32)
            nc.vector.memset(acc2[:], 0.0)
            with ExitStack() as l2s:
                # self-loop init: transpose h2_own chunks into acc2
                htp = l2s.enter_context(tc.tile_pool(name="htp", bufs=2))
                for q in range(4):
                    rows = H3PAD if q == 3 else AGSZ[q]
                    hT = htp.tile([D1, H3PAD], f16, tag="hT")
                    nc.sync.dma_start(hT[:, :rows], h2_own[q][:, :], transpose=True)
                    nc.vector.tensor_tensor(
                        acc2[:, AGOFF[q]:AGOFF[q] + AGSZ[q]],
                        acc2[:, AGOFF[q]:AGOFF[q] + AGSZ[q]],
                        hT[:, :AGSZ[q]],
                        mybir.AluOpType.add)

                gp2 = l2s.enter_context(tc.tile_pool(name="g2", bufs=8))
                pp2 = l2s.enter_context(tc.tile_pool(name="p2", bufs=3))
                psp2 = l2s.enter_context(tc.tile_pool(name="ps2", bufs=3, space="PSUM"))
                op = l2s.enter_context(tc.tile_pool(name="ob", bufs=4))
                pso = l2s.enter_context(tc.tile_pool(name="pso", bufs=2, space="PSUM"))

                def l2_dense(w):
                    for mm in range(4 * w, min(4 * w + 4, MCHUNKS)):
                        M = min(128, NS - mm * 128)
                        ps3 = pso.tile([M, D2], f32, tag="pso")
                        nc.tensor.matmul(ps3[:], acc2[:, mm * 128:mm * 128 + M],
                                         w2_t[:], start=True, stop=False)
                        nc.tensor.matmul(ps3[:], ones_t[:, :M], b2_t[:],
                                         start=False, stop=True)
                        ot = op.tile([M, D2], f32, tag="ot")
                        nc.scalar.activation(ot[:], ps3[:], Copy)
                        nc.sync.dma_start(out_d[mm * 128:mm * 128 + M, :], ot[:])

                ncalls = 0
                for k in range(NSRC):
                    Tk = meta2["Tk"][k]
                    segd = dict(meta2["segs"][k])
                    src_ap = bass.AP(hf[k][:].tensor, 0,
                                     [[D1, NC * AGSZ[k]], [1, D1]])
                    idx_t, dst_t, dd_t = idx2_t[k], dst2_t[k], dd2_t[k]
                    jj = 0
                    gt = None
                    gts = None
                    P8 = None
                    for w in range(NW):
                        nchk = segd.get(w, 0)
                        if nchk:
                            ps = psp2.tile([D1, WIN], f32, tag="ps2")
                            for j in range(nchk):
                                g, slot = divmod(jj, GC)
                                if slot == 0:
                                    mlen = min(G, Tk - g * G)
                                    mc = mlen // 128
                                    gt = gp2.tile([128, GC, D1], f16, tag="gt2")
                                    nc.gpsimd.dma_gather(
                                        gt[:, :mc, :], src_ap,
                                        idx_t[:, g * (G // 16): (g * G + mlen) // 16],
                                        mlen, mlen, D1,
                                        elem_step=D1,
                                        queue_num=ncalls % NQ,
                                        single_packet=True,
                                    )
                                    ncalls += 1
                                    gts = gp2.tile([128, GC, D1], f16, tag="gts")
                                    nc.vector.tensor_tensor(
                                        gts[:, :mc, :], gt[:, :mc, :],
                                        bcast(dd_t[:, jj:jj + mc], mc, D1),
                                        mybir.AluOpType.mult)
                                    P8 = pp2.tile([128, GC, WIN], f16, tag="P2")
                                    nc.vector.tensor_tensor(
                                        P8[:, :mc, :], iota16[:, :mc, :],
                                        bcast(dst_t[:, jj:jj + mc], mc),
                                        mybir.AluOpType.is_equal)
                                nc.tensor.matmul(ps[:], gts[:, slot, :],
                                                 P8[:, slot, :],
                                                 start=(j == 0), stop=(j == nchk - 1))
                                jj += 1
                            nc.vector.tensor_tensor(
                                acc2[:, w * WIN:(w + 1) * WIN],
                                acc2[:, w * WIN:(w + 1) * WIN], ps[:],
                                mybir.AluOpType.add)
                        if k == NSRC - 1:
                            l2_dense(w)
                if dbg_stages:
                    nc.sync.dma_start(dacc2_d[:], acc2[:])

    nc.compile()
    return nc


# ----------------------------------------------------------------------------
# Entry point
# ----------------------------------------------------------------------------
def _ensure_axon_hooks_module():
    """bass_utils hard-imports antenv.axon_hooks when BASS_TRACE is set;
    provide a degradable stub if the image's antenv lacks it."""
    import types

    try:
        import antenv.axon_hooks  # noqa: F401
        return
    except ImportError:
        pass
    try:
        import antenv
    except ImportError:
        return
    mod = types.ModuleType("antenv.axon_hooks")
    mod._hook = None
    mod.set_axon_ntff_profile_hook = lambda h: setattr(mod, "_hook", h)
    mod.get_axon_ntff_profile_hook = lambda: mod._hook
    sys.modules["antenv.axon_hooks"] = mod
    antenv.axon_hooks = mod


def kernel(x, edge_index, W1, b1, W2, b2):
    _ensure_axon_hooks_module()
    from concourse import bass_utils

    meta1, meta2, in_maps = _preprocess(x, edge_index, W1, b1, W2, b2)
    nc = _build(meta1, meta2, debug=False)
    res = bass_utils.run_bass_kernel_spmd(nc, in_maps, core_ids=list(range(NC)))
    out = np.concatenate([r["out"] for r in res.results], axis=0)
    return out.astype(np.float32)


# revision 12
# speedup vs baseline: 1.3191x; 1.3191x over previous
"""Trainium2 Bass kernel for a 2-layer GCN (PyG GCNConv x2 with self-loops).

Reference (N=100000 nodes, E=1600000 edges, f32):
    A = D^-1/2 (Adj + I) D^-1/2
    h   = relu(A x W1 + b1)
    out = A h W2 + b2

Key factorization: A[d,s] = dis[d]*dis[s] (dis = deg^-1/2, deg incl the
self loop).  The host pre-scales x rows by dis[src]; the dis[dst] factor
is applied entirely in the dense phase as a per-partition (per-dest-row)
activation scale, with the bias pre-divided by dis via a 1/dis row that
replaces the all-ones row in the bias matmul.  The aggregation therefore
accumulates raw sums, the selection matrix P is a pure one-hot built with
a single vector op per gather tile, and layer 2 feeds gathered rows
straight into the PE.

Self-loop (+I) terms never enter the edge stream: layer 1 initializes the
accumulator with the host-transposed x_dis shard; layer 2 DMA-transposes
the h*dis shard written during the layer-1 dense phase.

Distribution: destination sharding (12500 nodes/core), WIN=256 dest
windows, edges bucketed host-side by (source-chunk k, window w), padded
to 128-edge chunks equalized across cores (SPMD).  Per chunk the tensor
engine computes PSUM[feat, 256] += gt[:,slot,:]^T @ P.  Layer-1 output
h*dis is AllGathered in 4 window-aligned chunks which double as layer-2
source chunks; the layer-2 dense transform + output stores stream inside
the last aggregation pass.
"""

import os
import sys

import numpy as np

for _p in ("/opt/trn_rl_repo", "/root/.axon_site/_ro/trn_rl_repo"):
    if os.path.isdir(_p) and _p not in sys.path:
        sys.path.insert(0, _p)

# ----------------------------------------------------------------------------
# Problem constants
# ----------------------------------------------------------------------------
N = 100000
NC = 8
NS = N // NC            # 12500 dest nodes per core
D0, D1, D2 = 64, 128, 256
WIN = 256               # dest window width
NW = (NS + WIN - 1) // WIN          # 49 windows per core
ACCW = NW * WIN                     # 12544 acc columns
NSRC = 4                # source chunks per layer
SC1 = N // NSRC         # 25000 rows per layer-1 source chunk (src % 4)
# layer-2 / AllGather chunks: window-aligned [12,12,12,13] windows
AGSZ = [12 * WIN, 12 * WIN, 12 * WIN, NS - 36 * WIN]  # 3072,3072,3072,3284
AGOFF = [0, 3072, 6144, 9216]
H3PAD = 3296            # h_own_3 rows padded to 16-multiple for dma transpose
G = 1024                # edges per dma_gather call (SWDGE ring capacity)
GC = G // 128
NQ = 4
MCHUNKS = (NS + 127) // 128         # 98 dense row-chunks
MPW = WIN // 128                    # dense row-chunks per window (2)


# ----------------------------------------------------------------------------
# Host-side preprocessing
# ----------------------------------------------------------------------------
def _plan_layer(src_all, dloc_all, core_all, src_chunk_of, src_idx_of):
    """Bucket per-core edges by (source-chunk k, dest-window w), pad each
    bucket to a 128-multiple equal across cores.  Returns (meta, per_core):
      meta = {"Tk": [...], "segs": [[(w, nchunks), ...] per k]}
      per_core[c] = {"idx": int16 [128, T/16] x4, "dst": f16 [128, T/128] x4}
    """
    counts = np.zeros((NC, NSRC, NW), dtype=np.int64)
    per_core = []
    for c in range(NC):
        sel = core_all == c
        src = src_all[sel]
        dloc = dloc_all[sel]
        k = src_chunk_of(src)
        w = dloc // WIN
        idxl = src_idx_of(src)
        order = np.lexsort((w, k))
        k, w, idxl, dloc = k[order], w[order], idxl[order], dloc[order]
        key = k * NW + w
        counts[c] = np.bincount(key, minlength=NSRC * NW).reshape(NSRC, NW)
        per_core.append((k, w, idxl, dloc, key))

    nch = (counts.max(axis=0) + 127) // 128
    Tk = (nch.sum(axis=1) * 128).astype(np.int64)
    segs = [[(int(w), int(nch[k, w])) for w in range(NW) if nch[k, w] > 0]
            for k in range(NSRC)]

    base = np.zeros((NSRC, NW), dtype=np.int64)
    for k in range(NSRC):
        base[k] = np.concatenate(([0], np.cumsum(nch[k] * 128)[:-1]))

    out = []
    for c in range(NC):
        k, w, idxl, dloc, key = per_core[c]
        cnt = counts[c].reshape(-1)
        starts = np.concatenate(([0], np.cumsum(cnt)[:-1]))
        pos_in_bucket = np.arange(len(key)) - starts[key]
        tgt = base.reshape(-1)[key] + pos_in_bucket
        arrs = {"idx": [], "dst": []}
        for kk in range(NSRC):
            T = int(Tk[kk])
            idx16 = np.zeros(T, dtype=np.int16)
            dwf = np.full(T, -1.0, dtype=np.float16)
            m = k == kk
            t = tgt[m]
            idx16[t] = idxl[m].astype(np.int16)
            dwf[t] = (dloc[m] - (w[m] * WIN)).astype(np.float16)
            arrs["idx"].append(np.ascontiguousarray(
                np.tile(idx16.reshape(T // 16, 16).T, (8, 1))))
            arrs["dst"].append(np.ascontiguousarray(dwf.reshape(T // 128, 128).T))
        out.append(arrs)
    return {"Tk": [int(t) for t in Tk], "segs": segs}, out


def _preprocess(x, edge_index, W1, b1, W2, b2):
    row = np.asarray(edge_index[0], dtype=np.int64)
    col = np.asarray(edge_index[1], dtype=np.int64)
    deg = (np.bincount(col, minlength=N) + 1).astype(np.float32)
    dis = (1.0 / np.sqrt(deg)).astype(np.float32)

    core = (col // NS).astype(np.int64)
    dloc = col - core * NS

    meta1, arrs1 = _plan_layer(
        row, dloc, core,
        src_chunk_of=lambda s: s % NSRC,
        src_idx_of=lambda s: s // NSRC,
    )

    def chunk2_of(s):
        return np.minimum((s % NS) // AGSZ[0], 3)

    def idx2_of(s):
        c = s // NS
        r = s % NS
        q = np.minimum(r // AGSZ[0], 3)
        szq = np.asarray(AGSZ, dtype=np.int64)[q]
        offq = np.asarray(AGOFF, dtype=np.int64)[q]
        return c * szq + (r - offq)

    meta2, arrs2 = _plan_layer(row, dloc, core, chunk2_of, idx2_of)

    x = np.asarray(x, dtype=np.float32)
    x_dis = np.ascontiguousarray(x * dis[:, None])

    shared = {
        "x_dis": x_dis,
        "W1": np.ascontiguousarray(np.asarray(W1, dtype=np.float32)),
        "b1": np.ascontiguousarray(np.asarray(b1, dtype=np.float32).reshape(1, D1)),
        "W2": np.ascontiguousarray(np.asarray(W2, dtype=np.float32)),
        "b2": np.ascontiguousarray(np.asarray(b2, dtype=np.float32).reshape(1, D2)),
    }
    in_maps = []
    for c in range(NC):
        m = dict(shared)
        lo, hi = c * NS, (c + 1) * NS
        m["xT"] = np.ascontiguousarray(x_dis[lo:hi].T)          # [64, NS]
        dv = np.zeros(128 * MCHUNKS, dtype=np.float32)
        dv[:NS] = dis[lo:hi]
        m["disw"] = np.ascontiguousarray(dv.reshape(MCHUNKS, 128).T)  # [128, 98]
        di = np.zeros(MCHUNKS * 128, dtype=np.float32)
        di[:NS] = 1.0 / dis[lo:hi]
        m["disinv"] = np.ascontiguousarray(di.reshape(MCHUNKS, 128))  # [98, 128]
        for kk in range(NSRC):
            m[f"idx1_{kk}"] = arrs1[c]["idx"][kk]
            m[f"dst1_{kk}"] = arrs1[c]["dst"][kk]
            m[f"idx2_{kk}"] = arrs2[c]["idx"][kk]
            m[f"dst2_{kk}"] = arrs2[c]["dst"][kk]
        in_maps.append(m)
    return meta1, meta2, in_maps


# ----------------------------------------------------------------------------
# Device program
# ----------------------------------------------------------------------------
def _build(meta1, meta2, debug=False, dbg_stages=False):
    from contextlib import ExitStack

    import concourse.bacc as bacc
    import concourse.bass as bass
    import concourse.mybir as mybir
    import concourse.tile as tile

    f32, f16, i16 = mybir.dt.float32, mybir.dt.float16, mybir.dt.int16
    Relu = mybir.ActivationFunctionType.Relu
    Copy = mybir.ActivationFunctionType.Copy

    nc = bacc.Bacc("TRN2", target_bir_lowering=False, debug=debug,
                   num_devices=NC, num_swdge_queues=NQ)

    x_d = nc.dram_tensor("x_dis", [N, D0], f32, kind="ExternalInput")
    xT_d = nc.dram_tensor("xT", [D0, NS], f32, kind="ExternalInput")
    disw_d = nc.dram_tensor("disw", [128, MCHUNKS], f32, kind="ExternalInput")
    disinv_d = nc.dram_tensor("disinv", [MCHUNKS, 128], f32, kind="ExternalInput")
    w1_d = nc.dram_tensor("W1", [D0, D1], f32, kind="ExternalInput")
    b1_d = nc.dram_tensor("b1", [1, D1], f32, kind="ExternalInput")
    w2_d = nc.dram_tensor("W2", [D1, D2], f32, kind="ExternalInput")
    b2_d = nc.dram_tensor("b2", [1, D2], f32, kind="ExternalInput")

    idx1_d, dst1_d, idx2_d, dst2_d = [], [], [], []
    for k in range(NSRC):
        T1, T2 = meta1["Tk"][k], meta2["Tk"][k]
        idx1_d.append(nc.dram_tensor(f"idx1_{k}", [128, T1 // 16], i16, kind="ExternalInput"))
        dst1_d.append(nc.dram_tensor(f"dst1_{k}", [128, T1 // 128], f16, kind="ExternalInput"))
        idx2_d.append(nc.dram_tensor(f"idx2_{k}", [128, T2 // 16], i16, kind="ExternalInput"))
        dst2_d.append(nc.dram_tensor(f"dst2_{k}", [128, T2 // 128], f16, kind="ExternalInput"))

    h_own = [nc.dram_tensor(f"h_own{q}", [H3PAD if q == 3 else AGSZ[q], D1],
                            f16, kind="Internal") for q in range(4)]
    hf = [nc.dram_tensor(f"hf{q}", [NC * AGSZ[q], D1], f16, kind="Internal",
                         addr_space="Shared") for q in range(4)]
    out_d = nc.dram_tensor("out", [NS, D2], f32, kind="ExternalOutput")
    if dbg_stages:
        dacc1_d = nc.dram_tensor("dacc1", [D0, ACCW], f32, kind="ExternalOutput")
        dh_d = nc.dram_tensor("dh", [NS, D1], f16, kind="ExternalOutput")
        dacc2_d = nc.dram_tensor("dacc2", [D1, ACCW], f32, kind="ExternalOutput")

    def bcast(col_slice, mc, width=WIN):
        return bass.AP(col_slice.tensor, col_slice.offset,
                       [list(col_slice.ap[0]), [1, mc], [0, width]])

    with tile.TileContext(nc) as tc:
        with ExitStack() as top:
            const = top.enter_context(tc.tile_pool(name="const", bufs=1))
            w1_t = const.tile([D0, D1], f32)
            nc.sync.dma_start(w1_t[:], w1_d[:])
            b1_t = const.tile([1, D1], f32)
            nc.sync.dma_start(b1_t[:], b1_d[:])
            w2_t = const.tile([D1, D2], f32)
            nc.sync.dma_start(w2_t[:], w2_d[:])
            b2_t = const.tile([1, D2], f32)
            nc.sync.dma_start(b2_t[:], b2_d[:])
            disw_t = const.tile([128, MCHUNKS], f32)
            nc.sync.dma_start(disw_t[:], disw_d[:])

            iota16 = const.tile([128, GC, WIN], f16)
            nc.gpsimd.iota(iota16[:], pattern=[[0, GC], [1, WIN]], base=0,
                           channel_multiplier=0,
                           allow_small_or_imprecise_dtypes=True)

            accp = top.enter_context(tc.tile_pool(name="acc", bufs=1))

            # preload all layer-2 meta on the Activation HWDGE queue
            mp2 = top.enter_context(tc.tile_pool(name="meta2", bufs=1))
            idx2_t, dst2_t = [], []
            for k in range(NSRC):
                T2 = meta2["Tk"][k]
                t = mp2.tile([128, T2 // 16], i16, tag=f"idx2_{k}")
                nc.scalar.dma_start(t[:], idx2_d[k][:])
                idx2_t.append(t)
                t = mp2.tile([128, T2 // 128], f16, tag=f"dst2_{k}")
                nc.scalar.dma_start(t[:], dst2_d[k][:])
                dst2_t.append(t)

            # =========== Layer 1 ===========
            with ExitStack() as l1s:
                acc1p = l1s.enter_context(tc.tile_pool(name="acc1", bufs=1))
                acc1 = acc1p.tile([D0, ACCW], f32)
                nc.vector.memset(acc1[:, NS:], 0.0)
                nc.sync.dma_start(acc1[:, :NS], xT_d[:])   # self-loop init

                mp = l1s.enter_context(tc.tile_pool(name="meta1", bufs=2))
                gp = l1s.enter_context(tc.tile_pool(name="g1", bufs=8))
                pp = l1s.enter_context(tc.tile_pool(name="p1", bufs=6))
                psp = l1s.enter_context(tc.tile_pool(name="ps1", bufs=4, space="PSUM"))
                hp = l1s.enter_context(tc.tile_pool(name="hb", bufs=4))
                dvp = l1s.enter_context(tc.tile_pool(name="dv1", bufs=4))
                psb = l1s.enter_context(tc.tile_pool(name="psb", bufs=2, space="PSUM"))

                xb = x_d[:]
                x_srcs = [bass.AP(xb.tensor, k * D0, [[NSRC * D0, SC1], [1, D0]])
                          for k in range(NSRC)]

                def l1_dense(w):
                    q = min(w // 12, 3)
                    for mm in range(MPW * w, min(MPW * w + MPW, MCHUNKS)):
                        M = min(128, NS - mm * 128)
                        ps2 = psb.tile([M, D1], f32, tag="psb")
                        nc.tensor.matmul(ps2[:], acc1[:, mm * 128:mm * 128 + M],
                                         w1_t[:], start=True, stop=False)
                        div = dvp.tile([1, M], f32, tag="dv")
                        nc.scalar.dma_start(div[:], disinv_d[mm:mm + 1, :M])
                        nc.tensor.matmul(ps2[:], div[:],
                                         b1_t[:], start=False, stop=True)
                        ht = hp.tile([M, D1], f16, tag="ht")
                        nc.scalar.activation(ht[:], ps2[:], Relu,
                                             scale=disw_t[:M, mm:mm + 1])
                        hts = hp.tile([M, D1], f16, tag="hts")
                        nc.scalar.activation(hts[:], ht[:], Copy,
                                             scale=disw_t[:M, mm:mm + 1])
                        lo = mm * 128 - AGOFF[q]
                        nc.sync.dma_start(h_own[q][lo:lo + M, :], hts[:])

                ncalls = 0
                for k in range(NSRC):
                    Tk = meta1["Tk"][k]
                    segd = dict(meta1["segs"][k])
                    idx_t = mp.tile([128, Tk // 16], i16, tag="idx1")
                    nc.scalar.dma_start(idx_t[:], idx1_d[k][:])
                    dst_t = mp.tile([128, Tk // 128], f16, tag="dst1")
                    nc.scalar.dma_start(dst_t[:], dst1_d[k][:])
                    jj = 0
                    gt = None
                    gt16 = None
                    P8 = None
                    for w in range(NW):
                        nchk = segd.get(w, 0)
                        if nchk:
                            ps = psp.tile([D0, WIN], f32, tag="ps1")
                            for j in range(nchk):
                                g, slot = divmod(jj, GC)
                                if slot == 0:
                                    mlen = min(G, Tk - g * G)
                                    mc = mlen // 128
                                    gt = gp.tile([128, GC, D0], f32, tag="gt32")
                                    nc.gpsimd.dma_gather(
                                        gt[:, :mc, :], x_srcs[k],
                                        idx_t[:, g * (G // 16): (g * G + mlen) // 16],
                                        mlen, mlen, D0,
                                        elem_step=NSRC * D0,
                                        queue_num=ncalls % NQ,
                                        single_packet=True,
                                    )
                                    ncalls += 1
                                    gt16 = gp.tile([128, GC, D0], f16, tag="gt16")
                                    nc.scalar.activation(gt16[:, :mc, :],
                                                         gt[:, :mc, :], Copy)
                                    P8 = pp.tile([128, GC, WIN], f16, tag="P1")
                                    nc.vector.tensor_tensor(
                                        P8[:, :mc, :], iota16[:, :mc, :],
                                        bcast(dst_t[:, jj:jj + mc], mc),
                                        mybir.AluOpType.is_equal)
                                nc.tensor.matmul(ps[:], gt16[:, slot, :],
                                                 P8[:, slot, :],
                                                 start=(j == 0), stop=(j == nchk - 1))
                                jj += 1
                            nc.vector.tensor_tensor(
                                acc1[:, w * WIN:(w + 1) * WIN],
                                acc1[:, w * WIN:(w + 1) * WIN], ps[:],
                                mybir.AluOpType.add)
                        if k == NSRC - 1:
                            l1_dense(w)
                for q in range(4):
                    nc.gpsimd.collective_compute(
                        "AllGather", mybir.AluOpType.bypass,
                        replica_groups=[list(range(NC))],
                        ins=[h_own[q][:AGSZ[q], :]],
                        outs=[hf[q][:, :]],
                    )
                if dbg_stages:
                    nc.sync.dma_start(dacc1_d[:], acc1[:])
                    for q in range(4):
                        nc.sync.dma_start(
                            dh_d[AGOFF[q]:AGOFF[q] + AGSZ[q], :],
                            h_own[q][:AGSZ[q], :])

            # =========== Layer 2 ===========
            acc2 = accp.tile([D1, ACCW], f32)
            nc.vector.memset(acc2[:], 0.0)
            with ExitStack() as l2s:
                # self-loop init: transpose h_own chunks into acc2
                htp = l2s.enter_context(tc.tile_pool(name="htp", bufs=2))
                for q in range(4):
                    rows = H3PAD if q == 3 else AGSZ[q]
                    hT = htp.tile([D1, H3PAD], f16, tag="hT")
                    nc.sync.dma_start(hT[:, :rows], h_own[q][:, :], transpose=True)
                    nc.vector.tensor_tensor(
                        acc2[:, AGOFF[q]:AGOFF[q] + AGSZ[q]],
                        acc2[:, AGOFF[q]:AGOFF[q] + AGSZ[q]],
                        hT[:, :AGSZ[q]],
                        mybir.AluOpType.add)

                gp2 = l2s.enter_context(tc.tile_pool(name="g2", bufs=8))
                pp2 = l2s.enter_context(tc.tile_pool(name="p2", bufs=6))
                psp2 = l2s.enter_context(tc.tile_pool(name="ps2", bufs=4, space="PSUM"))
                op = l2s.enter_context(tc.tile_pool(name="ob", bufs=4))
                dvp2 = l2s.enter_context(tc.tile_pool(name="dv2", bufs=4))
                pso = l2s.enter_context(tc.tile_pool(name="pso", bufs=2, space="PSUM"))

                def l2_dense(w):
                    for mm in range(MPW * w, min(MPW * w + MPW, MCHUNKS)):
                        M = min(128, NS - mm * 128)
                        ps3 = pso.tile([M, D2], f32, tag="pso")
                        nc.tensor.matmul(ps3[:], acc2[:, mm * 128:mm * 128 + M],
                                         w2_t[:], start=True, stop=False)
                        div = dvp2.tile([1, M], f32, tag="dv")
                        nc.scalar.dma_start(div[:], disinv_d[mm:mm + 1, :M])
                        nc.tensor.matmul(ps3[:], div[:],
                                         b2_t[:], start=False, stop=True)
                        ot = op.tile([M, D2], f32, tag="ot")
                        nc.scalar.activation(ot[:], ps3[:], Copy,
                                             scale=disw_t[:M, mm:mm + 1])
                        nc.sync.dma_start(out_d[mm * 128:mm * 128 + M, :], ot[:])

                ncalls = 0
                for k in range(NSRC):
                    Tk = meta2["Tk"][k]
                    segd = dict(meta2["segs"][k])
                    src_ap = bass.AP(hf[k][:].tensor, 0,
                                     [[D1, NC * AGSZ[k]], [1, D1]])
                    idx_t, dst_t = idx2_t[k], dst2_t[k]
                    jj = 0
                    gt = None
                    P8 = None
                    for w in range(NW):
                        nchk = segd.get(w, 0)
                        if nchk:
                            ps = psp2.tile([D1, WIN], f32, tag="ps2")
                            for j in range(nchk):
                                g, slot = divmod(jj, GC)
                                if slot == 0:
                                    mlen = min(G, Tk - g * G)
                                    mc = mlen // 128
                                    gt = gp2.tile([128, GC, D1], f16, tag="gt2")
                                    nc.gpsimd.dma_gather(
                                        gt[:, :mc, :], src_ap,
                                        idx_t[:, g * (G // 16): (g * G + mlen) // 16],
                                        mlen, mlen, D1,
                                        elem_step=D1,
                                        queue_num=ncalls % NQ,
                                        single_packet=True,
                                    )
                                    ncalls += 1
                                    P8 = pp2.tile([128, GC, WIN], f16, tag="P2")
                                    nc.vector.tensor_tensor(
                                        P8[:, :mc, :], iota16[:, :mc, :],
                                        bcast(dst_t[:, jj:jj + mc], mc),
                                        mybir.AluOpType.is_equal)
                                nc.tensor.matmul(ps[:], gt[:, slot, :],
                                                 P8[:, slot, :],
                                                 start=(j == 0), stop=(j == nchk - 1))
                                jj += 1
                            nc.vector.tensor_tensor(
                                acc2[:, w * WIN:(w + 1) * WIN],
                                acc2[:, w * WIN:(w + 1) * WIN], ps[:],
                                mybir.AluOpType.add)
                        if k == NSRC - 1:
                            l2_dense(w)
                if dbg_stages:
                    nc.sync.dma_start(dacc2_d[:], acc2[:])

    nc.compile()
    return nc


# ----------------------------------------------------------------------------
# Entry point
# ----------------------------------------------------------------------------
def _ensure_axon_hooks_module():
    """bass_utils hard-imports antenv.axon_hooks when BASS_TRACE is set;
    provide a degradable stub if the image's antenv lacks it."""
    import types

    try:
        import antenv.axon_hooks  # noqa: F401
        return
    except ImportError:
        pass
    try:
        import antenv
    except ImportError:
        return
    mod = types.ModuleType("antenv.axon_hooks")
    mod._hook = None
    mod.set_axon_ntff_profile_hook = lambda h: setattr(mod, "_hook", h)
    mod.get_axon_ntff_profile_hook = lambda: mod._hook
    sys.modules["antenv.axon_hooks"] = mod
    antenv.axon_hooks = mod


def kernel(x, edge_index, W1, b1, W2, b2):
    _ensure_axon_hooks_module()
    from concourse import bass_utils

    meta1, meta2, in_maps = _preprocess(x, edge_index, W1, b1, W2, b2)
    nc = _build(meta1, meta2, debug=False)
    res = bass_utils.run_bass_kernel_spmd(nc, in_maps, core_ids=list(range(NC)))
    out = np.concatenate([r["out"] for r in res.results], axis=0)
    return out.astype(np.float32)
